# revision 4
# baseline (speedup 1.0000x reference)
"""Simple-HGN (2-layer edge-typed GAT + link-pred head) on 8 trn2 NeuronCores.

Strategy (see spec sharding_hint): destinations are sharded across the 8
cores; edges are partitioned by destination owner so segment softmax /
segment sums are core-local; the per-layer node feature tables
([feat | el] rows) are exchanged with AllGather; small weights replicated.

Device-side structure (identical SPMD program on all cores; all per-core
variation lives in input data):
  - gather table rows: [feat(128) | el(4) | pad] fp32, 192-elem (768B) stride.
  - 4 gather "groups" = row ranges <= 32767 (dma_gather idx is int16),
    issued on 4 SWDGE queues.
  - destination slots: B blocks x 128 slots per core. Per (block, group):
    TBG=2 tiles of <=128 edges, segments never split across tiles.
  - per tile: one-hot [edge,slot] + transposed one-hot via DVE is_equal;
    er-expansion matmul; fused aggregation+denominator matmul with
    rhs [V | a] where V = a * feat(src), a = exp(leaky(el+er+ee)).
  - per block: x = agg * recip(denom + 1e-16) + resid (+relu for layer 1).
  - Head: pair gathers from the AllGather'd final features + tiny matmuls.
"""

import os
import numpy as np

# ---------------------------------------------------------------- constants
N_CORES = 8
HEADS, HID, IN_CH, EDGE_CH = 4, 32, 128, 32
D = HEADS * HID            # 128
TROW = 192                 # table row stride in f32 elems (768B)
G = 4                      # gather groups
TBG = 2                    # tiles per (block, group)
SBB = 4                    # blocks per superblock (gather batch)
BT = TBG * SBB             # tiles per gather batch per group (8 -> 1024 idxs)
NIDX = BT * 128            # idxs per gather
PAD_EE = -500.0            # pad-edge ee -> exp(leaky(PAD_EE)) == 0

_NC_CACHE = {}


# ------------------------------------------------------------ host planning
def _fold_weights(W, al, ar):
    Wf = W.astype(np.float64).reshape(W.shape[0], HEADS, HID)
    Wal = np.einsum("chd,hd->ch", Wf, al.astype(np.float64)).astype(np.float32)
    War = np.einsum("chd,hd->ch", Wf, ar.astype(np.float64)).astype(np.float32)
    return Wal, War


def _ee_table(eemb, We, ae):
    ef = (eemb.astype(np.float64) @ We.astype(np.float64)).reshape(
        eemb.shape[0], HEADS, EDGE_CH)
    return np.einsum("thd,hd->th", ef, ae.astype(np.float64)).astype(np.float32)


def _wrap_idx(flat):
    """[.., n] int -> [.., 128, n//16] int16 (wrap 16, replicate 8x)."""
    n = flat.shape[-1]
    w = flat.reshape(flat.shape[:-1] + (n // 16, 16))
    w = np.swapaxes(w, -1, -2)  # [.., 16, n//16]
    reps = (1,) * (w.ndim - 2) + (8, 1)
    return np.ascontiguousarray(np.tile(w, reps)).astype(np.int16)


def _balanced_split(deg, n_parts):
    """Split positions [0..len(deg)) into n_parts contiguous ranges with
    roughly equal sum(deg). Returns boundary list of len n_parts+1."""
    cum = np.cumsum(deg)
    total = int(cum[-1]) if len(cum) else 0
    bounds = [0]
    for c in range(1, n_parts):
        t = total * c / n_parts
        bounds.append(int(np.searchsorted(cum, t)))
    bounds.append(len(deg))
    # enforce monotone
    for i in range(1, len(bounds)):
        bounds[i] = max(bounds[i], bounds[i - 1])
    return bounds


class _LayerPlan:
    """Per-layer, per-core tiling plan + streams."""

    def __init__(self, n_dst, n_edges, B):
        self.B = B  # blocks (pre-common-pad)
        self.slot_of = np.full(n_dst, -1, np.int64)     # core-local dst -> slot
        self.edge_tile = np.zeros((n_edges, 4), np.int64)  # (b, g, t, fill)


def _assemble_core(dst_pos, e_dst_pos, e_grp, tbg):
    """Greedy block/tile assembly for one core.

    dst_pos: core-local dst indices 0..nd-1 (sorted unique positions).
    e_dst_pos: per edge core-local dst position (edges sorted by (dst, grp)).
    e_grp: per edge group.
    Returns _LayerPlan (B, slot_of, edge_tile).
    """
    nd = len(dst_pos)
    ne = len(e_dst_pos)
    # per (dst, group) edge counts and start offsets (edges sorted by dst,grp)
    cnt = np.zeros((nd, G), np.int64)
    np.add.at(cnt, (e_dst_pos, e_grp), 1)
    starts = np.zeros((nd, G), np.int64)
    flat = cnt.reshape(-1)
    starts.reshape(-1)[1:] = np.cumsum(flat)[:-1]

    plan = _LayerPlan(nd, ne, 0)
    b = 0
    npos = 0
    fill = [0] * G
    cur = [0] * G
    for p in range(nd):
        c = cnt[p]
        ok = npos < 128
        if ok:
            for g in range(G):
                if c[g] == 0:
                    continue
                if fill[g] + c[g] > 128 and cur[g] + 1 >= tbg:
                    ok = False
                    break
        if not ok:
            b += 1
            npos = 0
            fill = [0] * G
            cur = [0] * G
        plan.slot_of[p] = b * 128 + npos
        npos += 1
        for g in range(G):
            if c[g] == 0:
                continue
            if fill[g] + c[g] > 128:
                cur[g] += 1
                fill[g] = 0
            s = starts[p, g]
            plan.edge_tile[s:s + c[g], 0] = b
            plan.edge_tile[s:s + c[g], 1] = g
            plan.edge_tile[s:s + c[g], 2] = cur[g]
            plan.edge_tile[s:s + c[g], 3] = fill[g] + np.arange(c[g])
            fill[g] += c[g]
    plan.B = b + 1
    return plan


def _layer_streams(plan, B_pad, e_srcloc, e_ee, e_dloc):
    """Build idx / dcol / drow / ee stream arrays for one core & layer.

    e_srcloc: group-local gather row per edge (int), e_ee: [ne,4] f32,
    e_dloc: slot - 128*b per edge.
    """
    NSB = B_pad // SBB
    idx = np.zeros((NSB, G, NIDX), np.int64)
    dcol = np.zeros((NSB, G, 128, BT), np.float32)
    drow = np.zeros((NSB, G, BT, 128), np.float32)
    ee = np.full((NSB, G, 128, BT, 4), PAD_EE, np.float32)

    b = plan.edge_tile[:, 0]
    g = plan.edge_tile[:, 1]
    t = plan.edge_tile[:, 2]
    k = plan.edge_tile[:, 3]
    sb = b // SBB
    j = (b % SBB) * TBG + t
    i = j * 128 + k
    idx[sb, g, i] = e_srcloc
    dcol[sb, g, k, j] = e_dloc
    drow[sb, g, j, k] = e_dloc
    ee[sb, g, k, j, :] = e_ee
    return _wrap_idx(idx.reshape(NSB, G, NIDX)), dcol, drow, ee


def _prep(inputs):
    """Full host preprocessing -> plan dict (structure consts + per-core arrays)."""
    ei = np.asarray(inputs["edge_index"])
    src_all = np.asarray(ei[0]).astype(np.int64)
    dst_all = np.asarray(ei[1]).astype(np.int64)
    et_all = np.asarray(inputs["edge_type"]).astype(np.int64)
    n_id = np.asarray(inputs["n_id"]).astype(np.int64)
    neg = int(np.asarray(inputs["neg_sample_num"]))
    id_emb = np.asarray(inputs["id_emb"]).astype(np.float32)
    N = id_emb.shape[0]

    Wal0, War0 = _fold_weights(inputs["W0"], inputs["al0"], inputs["ar0"])
    Wal1, War1 = _fold_weights(inputs["W1"], inputs["al1"], inputs["ar1"])
    ee0 = _ee_table(np.asarray(inputs["eemb0"]), np.asarray(inputs["We0"]),
                    np.asarray(inputs["ae0"]))
    ee1 = _ee_table(np.asarray(inputs["eemb1"]), np.asarray(inputs["We1"]),
                    np.asarray(inputs["ae1"]))

    bsz = len(n_id) // (2 + neg)

    # ---- pruning ----
    T2 = np.unique(n_id)
    m2 = np.isin(dst_all, T2)
    S2 = np.unique(src_all[m2])
    T1 = np.union1d(T2, S2)
    m1 = np.isin(dst_all, T1)
    S1 = np.unique(src_all[m1])

    # ---- L1 structures ----
    # gather table rows = S1 positions, sharded into 8 producer chunks
    PROWS = -(-len(S1) // N_CORES)
    PROWS = -(-PROWS // 128) * 128
    while (2 * PROWS) > 32767:
        raise ValueError("group too large")
    GR1 = 2 * PROWS  # group rows (2 producers per group)

    src1 = src_all[m1]
    dst1 = dst_all[m1]
    et1 = et_all[m1]
    # per-edge gather position & group
    e_spos1 = np.searchsorted(S1, src1)
    # producer chunk of a src = spos // PROWS_real ... chunks are equal PROWS
    # slices of the padded position space: position p lives in chunk p//PROWS?
    # We place S1 row r at producer (r // ceil(|S1|/8)) with per-producer
    # compact rows; simpler: global padded position = r + (r // RP) * (PROWS - RP)
    RP = -(-len(S1) // N_CORES)  # real rows per producer (last may be short)
    prod = e_spos1 // RP
    gpos1 = prod * PROWS + (e_spos1 - prod * RP)
    e_grp1 = gpos1 // GR1
    e_sloc1 = gpos1 - e_grp1 * GR1

    # dst cores: contiguous, edge-balanced over T1
    t1pos = np.searchsorted(T1, dst1)
    deg1 = np.bincount(t1pos, minlength=len(T1))
    cb1 = _balanced_split(deg1, N_CORES)

    # ---- per-core L1 assembly ----
    order1 = np.lexsort((e_grp1, t1pos))
    t1pos_s, grp1_s, sloc1_s = t1pos[order1], e_grp1[order1], e_sloc1[order1]
    et1_s, src1_s = et1[order1], src1[order1]
    plans1 = []
    for c in range(N_CORES):
        lo, hi = cb1[c], cb1[c + 1]
        emask = (t1pos_s >= lo) & (t1pos_s < hi)
        plans1.append((np.nonzero(emask)[0], lo, hi))
    cores1 = []
    B1 = 1
    for c in range(N_CORES):
        eidx, lo, hi = plans1[c]
        pl = _assemble_core(np.arange(hi - lo), t1pos_s[eidx] - lo,
                            grp1_s[eidx], TBG)
        cores1.append((pl, eidx, lo, hi))
        B1 = max(B1, pl.B)
    B1 = -(-B1 // SBB) * SBB
    NSB1 = B1 // SBB

    # global L1 slot of every T1 node (for L2 src mapping & residual/er)
    slot1_of_t1 = np.full(len(T1), -1, np.int64)
    for c in range(N_CORES):
        pl, eidx, lo, hi = cores1[c]
        slot1_of_t1[lo:hi] = c * B1 * 128 + pl.slot_of

    # ---- L2 structures ----
    src2 = src_all[m2]
    dst2 = dst_all[m2]
    et2 = et_all[m2]
    # gather position in table2 = global L1 slot of src (src in T1 since T1>=S2)
    gpos2 = slot1_of_t1[np.searchsorted(T1, src2)]
    NROW2 = N_CORES * B1 * 128
    GR2 = NROW2 // G
    assert GR2 % 128 == 0 and GR2 <= 32767, (NROW2, GR2)
    e_grp2 = gpos2 // GR2
    e_sloc2 = gpos2 - e_grp2 * GR2

    # L2 dst cores: T2 split at the same T1 ownership boundaries
    t2_in_t1 = np.searchsorted(T1, T2)
    t2_core = np.searchsorted(np.array(cb1[1:]), t2_in_t1, side="right")
    t2pos = np.searchsorted(T2, dst2)
    cores2 = []
    B2 = 1
    order2 = np.lexsort((e_grp2, t2pos))
    t2pos_s, grp2_s, sloc2_s = t2pos[order2], e_grp2[order2], e_sloc2[order2]
    et2_s = et2[order2]
    cb2 = [int(np.searchsorted(t2_core, c)) for c in range(N_CORES)] + [len(T2)]
    for c in range(N_CORES):
        lo, hi = cb2[c], cb2[c + 1]
        emask = (t2pos_s >= lo) & (t2pos_s < hi)
        eidx = np.nonzero(emask)[0]
        pl = _assemble_core(np.arange(hi - lo), t2pos_s[eidx] - lo,
                            grp2_s[eidx], TBG)
        cores2.append((pl, eidx, lo, hi))
        B2 = max(B2, pl.B)
    B2 = -(-B2 // SBB) * SBB
    NSB2 = B2 // SBB

    slot2_of_t2 = np.full(len(T2), -1, np.int64)
    for c in range(N_CORES):
        pl, eidx, lo, hi = cores2[c]
        slot2_of_t2[lo:hi] = c * B2 * 128 + pl.slot_of

    # ---- per-core input arrays ----
    per_core = []
    for c in range(N_CORES):
        pl1, eidx1, lo1, hi1 = cores1[c]
        pl2, eidx2, lo2, hi2 = cores2[c]

        idx1, dcol1, drow1, eearr1 = _layer_streams(
            pl1, B1, sloc1_s[eidx1], ee0[et1_s[eidx1]],
            (pl1.slot_of[t1pos_s[eidx1] - lo1] % 128).astype(np.float32))
        idx2, dcol2, drow2, eearr2 = _layer_streams(
            pl2, B2, sloc2_s[eidx2], ee1[et2_s[eidx2]],
            (pl2.slot_of[t2pos_s[eidx2] - lo2] % 128).astype(np.float32))

        # producer chunk sources (S1 rows of this producer)
        s1_lo, s1_hi = c * RP, min((c + 1) * RP, len(S1))
        embT_src = np.zeros((IN_CH, PROWS), np.float32)
        embT_src[:, : s1_hi - s1_lo] = id_emb[S1[s1_lo:s1_hi]].T

        # dst-side id_emb expanded to slot space
        embT_dst = np.zeros((IN_CH, B1 * 128), np.float32)
        sl = pl1.slot_of
        embT_dst[:, sl] = id_emb[T1[lo1:hi1]].T

        # L2 dst slabs: L1 slot (core-local) of each L2 dst slot
        d2idx = np.zeros((B2, 128), np.int64)
        g1 = slot1_of_t1[t2_in_t1[lo2:hi2]] - c * B1 * 128
        assert (g1 >= 0).all() and (g1 < B1 * 128).all()
        sl2 = pl2.slot_of
        d2idx[sl2 // 128, sl2 % 128] = g1
        d2w = _wrap_idx(d2idx.reshape(B2, 128))

        per_core.append(dict(
            idx1=idx1, dcol1=dcol1, drow1=drow1, ee1=eearr1,
            idx2=idx2, dcol2=dcol2, drow2=drow2, ee2=eearr2,
            embT_src=embT_src, embT_dst=embT_dst, d2idx=d2w,
        ))

    # ---- head pairs ----
    out_n = n_id[:bsz]
    pos_n = n_id[bsz:2 * bsz]
    neg_n = n_id[2 * bsz:]
    a_nodes = np.concatenate([out_n, np.repeat(out_n, neg)])
    b_nodes = np.concatenate([pos_n, neg_n])
    a_slot = slot2_of_t2[np.searchsorted(T2, a_nodes)]
    b_slot = slot2_of_t2[np.searchsorted(T2, b_nodes)]
    npairs = len(a_nodes)
    HB_total = -(-npairs // (128 * N_CORES)) * N_CORES
    HB = HB_total // N_CORES
    pad_n = HB_total * 128 - npairs
    a_slot = np.concatenate([a_slot, np.zeros(pad_n, np.int64)])
    b_slot = np.concatenate([b_slot, np.zeros(pad_n, np.int64)])
    a_slot = a_slot.reshape(N_CORES, HB * 128)
    b_slot = b_slot.reshape(N_CORES, HB * 128)
    for c in range(N_CORES):
        per_core[c]["hA"] = _wrap_idx(a_slot[c].reshape(HB, 128))
        per_core[c]["hB"] = _wrap_idx(b_slot[c].reshape(HB, 128))

    # ---- shared weight inputs ----
    f32 = np.float32
    shared = dict(
        Wsrc1=np.concatenate([np.asarray(inputs["W0"], f32), Wal0], axis=1),
        Wdst1=np.concatenate([np.asarray(inputs["Wres0"], f32), War0], axis=1),
        Wsrc2=np.concatenate([np.asarray(inputs["W1"], f32), Wal1,
                              np.asarray(inputs["Wres1"], f32), War1], axis=1),
        pW1a=np.asarray(inputs["pW1"], f32)[:D],
        pW1b=np.asarray(inputs["pW1"], f32)[D:],
        pW2=np.asarray(inputs["pW2"], f32),
        pb1=np.asarray(inputs["pb1"], f32).reshape(HID, 1),
        pb2=np.full((128, 1), float(np.asarray(inputs["pb2"])[0]), f32),
        iota_row=np.tile(np.arange(128, dtype=f32)[None, :], (128, 1)),
        iota_col=np.arange(128, dtype=f32).reshape(128, 1),
        ident=np.eye(128, dtype=f32),
    )

    struct = dict(PROWS=PROWS, GR1=GR1, B1=B1, NSB1=NSB1, B2=B2, NSB2=NSB2,
                  GR2=GR2, HB=HB)
    return dict(struct=struct, per_core=per_core, shared=shared,
                bsz=bsz, neg=neg, npairs=npairs)


# ------------------------------------------------------------- bass builder
def _build_nc(st, dbg=False):
    import concourse.bass as bass
    import concourse.bacc as bacc
    import concourse.mybir as mybir
    import concourse.tile as tile
    from concourse import library_config

    f32 = mybir.dt.float32
    i16 = mybir.dt.int16
    AF = mybir.ActivationFunctionType
    ALU = mybir.AluOpType

    PROWS, GR1, B1, NSB1 = st["PROWS"], st["GR1"], st["B1"], st["NSB1"]
    B2, NSB2, GR2, HB = st["B2"], st["NSB2"], st["GR2"], st["HB"]
    NROW1 = N_CORES * PROWS
    NROW2 = N_CORES * B1 * 128

    nc = bacc.Bacc("TRN2", num_devices=N_CORES, num_swdge_queues=G)

    # -------- dram tensors
    def din(name, shape, dt=f32):
        return nc.dram_tensor(name, shape, dt, kind="ExternalInput")

    idx1 = din("idx1", [NSB1, G, 128, NIDX // 16], i16)
    dcol1 = din("dcol1", [NSB1, G, 128, BT])
    drow1 = din("drow1", [NSB1, G, BT, 128])
    ee1 = din("ee1", [NSB1, G, 128, BT, 4])
    idx2 = din("idx2", [NSB2, G, 128, NIDX // 16], i16)
    dcol2 = din("dcol2", [NSB2, G, 128, BT])
    drow2 = din("drow2", [NSB2, G, BT, 128])
    ee2 = din("ee2", [NSB2, G, 128, BT, 4])
    embT_src = din("embT_src", [IN_CH, PROWS])
    embT_dst = din("embT_dst", [IN_CH, B1 * 128])
    d2idx = din("d2idx", [B2, 128, 8], i16)
    hA = din("hA", [HB, 128, 8], i16)
    hB = din("hB", [HB, 128, 8], i16)
    Wsrc1 = din("Wsrc1", [IN_CH, 132])
    Wdst1 = din("Wdst1", [IN_CH, 132])
    Wsrc2 = din("Wsrc2", [D, 264])
    pW1a = din("pW1a", [D, HID])
    pW1b = din("pW1b", [D, HID])
    pW2 = din("pW2", [HID, 1])
    pb1 = din("pb1", [HID, 1])
    pb2 = din("pb2", [128, 1])
    iota_row = din("iota_row", [128, 128])
    iota_col = din("iota_col", [128, 1])
    ident = din("ident", [128, 128])

    out_logits = nc.dram_tensor("out_logits", [HB * 128, 1], f32,
                                kind="ExternalOutput")
    if dbg:
        dbg_x1 = nc.dram_tensor("dbg_x1", [B1 * 128, D], f32,
                                kind="ExternalOutput")
        dbg_x2 = nc.dram_tensor("dbg_x2", [B2 * 128, D], f32,
                                kind="ExternalOutput")

    chunk1 = nc.dram_tensor("chunk1", [PROWS, TROW], f32)
    table1 = nc.dram_tensor("table1", [NROW1, TROW], f32, addr_space="Shared")
    chunk2 = nc.dram_tensor("chunk2", [B1 * 128, TROW], f32)
    table2 = nc.dram_tensor("table2", [NROW2, TROW], f32, addr_space="Shared")
    dst2t = nc.dram_tensor("dst2t", [B1 * 128, TROW], f32)
    x2chunk = nc.dram_tensor("x2chunk", [B2 * 128, D], f32)
    x2ag = nc.dram_tensor("x2ag", [N_CORES * B2 * 128, D], f32,
                          addr_space="Shared")

    rg = [list(range(N_CORES))]

    with tile.TileContext(nc) as tc:
        nc.gpsimd.load_library(library_config.mlp)
        import contextlib
        ctx = contextlib.ExitStack()
        cpool = ctx.enter_context(tc.tile_pool(name="const", bufs=1))
        slab = ctx.enter_context(tc.tile_pool(name="slab", bufs=1))
        sp = ctx.enter_context(tc.tile_pool(name="stream", bufs=4))
        gp = ctx.enter_context(tc.tile_pool(name="gbuf", bufs=6))
        ohp = ctx.enter_context(tc.tile_pool(name="oh", bufs=2 * G * BT + 2))
        ohtp = ctx.enter_context(tc.tile_pool(name="oht", bufs=10))
        vp = ctx.enter_context(tc.tile_pool(name="vsl", bufs=5))
        smp = ctx.enter_context(tc.tile_pool(name="small", bufs=10))
        pp_agg = ctx.enter_context(tc.tile_pool(name="ps_agg", bufs=3, space="PSUM"))
        pp_er = ctx.enter_context(tc.tile_pool(name="ps_er", bufs=2, space="PSUM"))
        pp_prod = ctx.enter_context(tc.tile_pool(name="ps_prod", bufs=2, space="PSUM"))
        pp_tp = ctx.enter_context(tc.tile_pool(name="ps_tp", bufs=1, space="PSUM"))

        # -------- consts to SBUF
        def const_tile(src_ap, shape, tag, dt=f32):
            t = cpool.tile(shape, dt, tag=tag)
            nc.sync.dma_start(out=t[:], in_=src_ap)
            return t

        iota_row_t = const_tile(iota_row[:], [128, 128], "c_ir")
        iota_col_t = const_tile(iota_col[:], [128, 1], "c_ic")
        ident_t = const_tile(ident[:], [128, 128], "c_id")
        Wsrc1_t = const_tile(Wsrc1[:], [IN_CH, 132], "c_w1")
        Wdst1_t = const_tile(Wdst1[:], [IN_CH, 132], "c_wd")
        Wsrc2_t = const_tile(Wsrc2[:], [D, 264], "c_w2")
        pW1a_t = const_tile(pW1a[:], [D, HID], "c_pa")
        pW1b_t = const_tile(pW1b[:], [D, HID], "c_pb")
        pW2_t = const_tile(pW2[:], [HID, 1], "c_p2")
        pb1_t = const_tile(pb1[:], [HID, 1], "c_b1")
        pb2_t = const_tile(pb2[:], [128, 1], "c_b2")

        # -------- persistent slabs
        er1_s = slab.tile([128, B1, 4], f32, tag="er1")
        rs1_s = slab.tile([128, B1, D], f32, tag="rs1")
        x1_s = slab.tile([128, B1, D], f32, tag="x1")
        er2_s = slab.tile([128, B2, 4], f32, tag="er2")
        rs2_s = slab.tile([128, B2, D], f32, tag="rs2")
        x2_s = slab.tile([128, B2, D], f32, tag="x2")

        # ---------------- produce layer 1 table ----------------
        for i in range(PROWS // 128):
            emb_t = sp.tile([128, 128], f32, tag="emb")
            nc.sync.dma_start(out=emb_t[:], in_=embT_src[:, i * 128:(i + 1) * 128])
            ps = pp_prod.tile([128, 264], f32, tag="prod")
            nc.tensor.matmul(out=ps[:, :132], lhsT=emb_t[:], rhs=Wsrc1_t[:],
                             start=True, stop=True)
            stg = sp.tile([128, 132], f32, tag="pstg")
            nc.vector.tensor_copy(out=stg[:], in_=ps[:, :132])
            nc.scalar.dma_start(out=chunk1[i * 128:(i + 1) * 128, 0:132],
                                in_=stg[:])
        # dst-side: resid + er slabs
        for b in range(B1):
            emb_t = sp.tile([128, 128], f32, tag="emb")
            nc.sync.dma_start(out=emb_t[:], in_=embT_dst[:, b * 128:(b + 1) * 128])
            ps = pp_prod.tile([128, 264], f32, tag="prod")
            nc.tensor.matmul(out=ps[:, :132], lhsT=emb_t[:], rhs=Wdst1_t[:],
                             start=True, stop=True)
            nc.vector.tensor_copy(out=rs1_s[:, b, :], in_=ps[:, 0:D])
            nc.vector.tensor_copy(out=er1_s[:, b, :], in_=ps[:, D:132])

        nc.gpsimd.collective_compute(
            "AllGather", ALU.bypass, replica_groups=rg,
            ins=[chunk1[:]], outs=[table1[:]])

        # ---------------- generic aggregation layer ----------------
        def agg_layer(NSB, idx_d, dcol_d, drow_d, ee_d, table_d, GRX,
                      er_slab, rs_slab, x_slab, relu):
            for sb in range(NSB):
                gbufs, vs, dcols = [], [], []
                ohs = [[None] * BT for _ in range(G)]
                er_pss = []
                for g in range(G):
                    it = sp.tile([128, NIDX // 16], i16, tag="idx")
                    nc.sync.dma_start(out=it[:], in_=idx_d[sb, g])
                    gb = gp.tile([128, BT, TROW], f32, tag="gb")
                    nc.gpsimd.dma_gather(
                        gb[:], table_d[GRX * g:GRX * (g + 1), :], it[:],
                        NIDX, NIDX, TROW, single_packet=False, queue_num=g)
                    gbufs.append(gb)
                    dc = sp.tile([128, BT], f32, tag="dcol")
                    nc.sync.dma_start(out=dc[:], in_=dcol_d[sb, g])
                    dcols.append(dc)
                    dr = sp.tile([128, BT, 128], f32, tag="drow")
                    drsrc = drow_d[sb, g]  # [BT, 128]
                    bcast = bass.AP(drsrc.tensor, drsrc.offset,
                                    [[0, 128]] + list(drsrc.ap))
                    nc.sync.dma_start(out=dr[:], in_=bcast)
                    eet = sp.tile([128, BT, 4], f32, tag="ee")
                    nc.sync.dma_start(out=eet[:], in_=ee_d[sb, g])

                    # per-tile OH/OHT + er matmul
                    er_ps = pp_er.tile([128, BT * 4], f32, tag="er")
                    for j in range(BT):
                        b = sb * SBB + j // TBG
                        oh = ohp.tile([128, 128], f32, tag="oh")
                        nc.vector.tensor_scalar(
                            out=oh[:], in0=iota_row_t[:], scalar1=dc[:, j:j + 1],
                            scalar2=None, op0=ALU.is_equal)
                        ohs[g][j] = oh
                        oht = ohtp.tile([128, 128], f32, tag="oht")
                        nc.vector.tensor_scalar(
                            out=oht[:], in0=dr[:, j, :], scalar1=iota_col_t[:],
                            scalar2=None, op0=ALU.is_equal)
                        nc.tensor.matmul(out=er_ps[:, 4 * j:4 * j + 4],
                                         lhsT=oht[:], rhs=er_slab[:, b, :],
                                         start=True, stop=True)
                    # batched logit pipeline
                    v = vp.tile([128, BT, 132], f32, tag="v")
                    vs.append(v)
                    nc.vector.tensor_tensor(
                        out=eet[:], in0=eet[:],
                        in1=gb[:, :, 128:132], op=ALU.add)
                    nc.vector.tensor_tensor(
                        out=v[:, :, 128:132], in0=eet[:],
                        in1=er_ps[:].rearrange("p (j h) -> p j h", h=4),
                        op=ALU.add)
                    nc.vector.tensor_scalar(
                        out=eet[:], in0=v[:, :, 128:132], scalar1=0.2,
                        scalar2=None, op0=ALU.mult)
                    nc.vector.tensor_tensor(
                        out=v[:, :, 128:132], in0=v[:, :, 128:132],
                        in1=eet[:], op=ALU.max)
                    nc.scalar.activation(out=v[:, :, 128:132],
                                         in_=v[:, :, 128:132], func=AF.Exp)
                    nc.vector.tensor_tensor(
                        out=v[:, :, 0:D].rearrange("p j (h d) -> p j h d", d=HID),
                        in0=gb[:, :, 0:D].rearrange("p j (h d) -> p j h d", d=HID),
                        in1=v[:, :, 128:132].to_broadcast([128, BT, 4, HID]),
                        op=ALU.mult)

                for bl in range(SBB):
                    b = sb * SBB + bl
                    agg = pp_agg.tile([128, 132], f32, tag="agg")
                    n_mm = G * TBG
                    m = 0
                    for g in range(G):
                        for t in range(TBG):
                            j = bl * TBG + t
                            nc.tensor.matmul(
                                out=agg[:], lhsT=ohs[g][j][:],
                                rhs=vs[g][:, j, :],
                                start=(m == 0), stop=(m == n_mm - 1))
                            m += 1
                    den = smp.tile([128, 4], f32, tag="den")
                    nc.vector.tensor_scalar(out=den[:], in0=agg[:, 128:132],
                                            scalar1=1e-16, scalar2=None,
                                            op0=ALU.add)
                    rec = smp.tile([128, 4], f32, tag="rec")
                    nc.vector.reciprocal(out=rec[:], in_=den[:])
                    xs = smp.tile([128, D], f32, tag="xstg")
                    nc.vector.tensor_tensor(
                        out=xs[:].rearrange("p (h d) -> p h d", d=HID),
                        in0=agg[:, 0:D].rearrange("p (h d) -> p h d", d=HID),
                        in1=rec[:].to_broadcast([128, 4, HID]), op=ALU.mult)
                    if relu:
                        nc.vector.tensor_tensor(out=xs[:], in0=xs[:],
                                                in1=rs_slab[:, b, :], op=ALU.add)
                        nc.scalar.activation(out=x_slab[:, b, :], in_=xs[:],
                                             func=AF.Relu)
                    else:
                        nc.vector.tensor_tensor(out=x_slab[:, b, :], in0=xs[:],
                                                in1=rs_slab[:, b, :], op=ALU.add)

        agg_layer(NSB1, idx1, dcol1, drow1, ee1, table1, GR1,
                  er1_s, rs1_s, x1_s, relu=True)
        if dbg:
            for b in range(B1):
                nc.scalar.dma_start(out=dbg_x1[b * 128:(b + 1) * 128, :],
                                    in_=x1_s[:, b, :])

        # ---------------- produce layer 2 table + dst slabs ----------------
        for b in range(B1):
            tp = pp_tp.tile([128, 128], f32, tag="tp")
            nc.tensor.transpose(out=tp[:], in_=x1_s[:, b, :], identity=ident_t[:])
            xt = sp.tile([128, 128], f32, tag="xt")
            nc.vector.tensor_copy(out=xt[:], in_=tp[:])
            ps = pp_prod.tile([128, 264], f32, tag="prod")
            nc.tensor.matmul(out=ps[:], lhsT=xt[:], rhs=Wsrc2_t[:],
                             start=True, stop=True)
            stg = sp.tile([128, 132], f32, tag="pstg")
            nc.vector.tensor_copy(out=stg[:], in_=ps[:, 0:132])
            nc.scalar.dma_start(out=chunk2[b * 128:(b + 1) * 128, 0:132],
                                in_=stg[:])
            stg2 = sp.tile([128, 132], f32, tag="pstg")
            nc.vector.tensor_copy(out=stg2[:], in_=ps[:, 132:264])
            nc.scalar.dma_start(out=dst2t[b * 128:(b + 1) * 128, 0:132],
                                in_=stg2[:])

        nc.gpsimd.collective_compute(
            "AllGather", ALU.bypass, replica_groups=rg,
            ins=[chunk2[:]], outs=[table2[:]])

        # L2 dst slabs via gather from dst2t
        for b2 in range(B2):
            it = sp.tile([128, 8], i16, tag="idx")
            nc.sync.dma_start(out=it[:], in_=d2idx[b2])
            db = gp.tile([128, 1, TROW], f32, tag="d2b")
            nc.gpsimd.dma_gather(db[:], dst2t[:], it[:], 128, 128, TROW,
                                 single_packet=False, queue_num=b2 % G)
            nc.vector.tensor_copy(out=rs2_s[:, b2, :], in_=db[:, 0, 0:D])
            nc.vector.tensor_copy(out=er2_s[:, b2, :], in_=db[:, 0, D:132])

        agg_layer(NSB2, idx2, dcol2, drow2, ee2, table2, GR2,
                  er2_s, rs2_s, x2_s, relu=False)
        if dbg:
            for b in range(B2):
                nc.scalar.dma_start(out=dbg_x2[b * 128:(b + 1) * 128, :],
                                    in_=x2_s[:, b, :])

        # ---------------- x2 AllGather + head ----------------
        for b2 in range(B2):
            nc.scalar.dma_start(out=x2chunk[b2 * 128:(b2 + 1) * 128, :],
                                in_=x2_s[:, b2, :])
        nc.gpsimd.collective_compute(
            "AllGather", ALU.bypass, replica_groups=rg,
            ins=[x2chunk[:]], outs=[x2ag[:]])

        for hb in range(HB):
            gtiles = []
            for nm, idxd in (("A", hA), ("B", hB)):
                it = sp.tile([128, 8], i16, tag="idx")
                nc.sync.dma_start(out=it[:], in_=idxd[hb])
                gt = gp.tile([128, 1, D], f32, tag="hg")
                nc.gpsimd.dma_gather(gt[:], x2ag[:], it[:], 128, 128, D,
                                     single_packet=False,
                                     queue_num=(2 * hb + (nm == "B")) % G)
                tp = pp_tp.tile([128, 128], f32, tag="tp")
                nc.tensor.transpose(out=tp[:], in_=gt[:, 0, :],
                                    identity=ident_t[:])
                ts = sp.tile([128, 128], f32, tag="xt")
                nc.vector.tensor_copy(out=ts[:], in_=tp[:])
                gtiles.append(ts)
            zp = pp_prod.tile([HID, 128], f32, tag="prod")
            nc.tensor.matmul(out=zp[:], lhsT=pW1a_t[:], rhs=gtiles[0][:],
                             start=True, stop=False)
            nc.tensor.matmul(out=zp[:], lhsT=pW1b_t[:], rhs=gtiles[1][:],
                             start=False, stop=True)
            zb = sp.tile([HID, 128], f32, tag="zb")
            nc.vector.tensor_scalar(out=zb[:], in0=zp[:], scalar1=pb1_t[:],
                                    scalar2=None, op0=ALU.add)
            zt = sp.tile([HID, 128], f32, tag="zt")
            nc.vector.tensor_scalar(out=zt[:], in0=zb[:], scalar1=0.2,
                                    scalar2=None, op0=ALU.mult)
            zs = sp.tile([HID, 128], f32, tag="zs")
            nc.vector.tensor_tensor(out=zs[:], in0=zb[:], in1=zt[:],
                                    op=ALU.max)
            op = pp_er.tile([128, 1], f32, tag="er")
            nc.tensor.matmul(out=op[:], lhsT=zs[:], rhs=pW2_t[:],
                             start=True, stop=True)
            ot = smp.tile([128, 1], f32, tag="ot")
            nc.vector.tensor_tensor(out=ot[:], in0=op[:], in1=pb2_t[:],
                                    op=ALU.add)
            nc.sync.dma_start(out=out_logits[hb * 128:(hb + 1) * 128, :],
                              in_=ot[:])
        ctx.close()
    nc.compile()
    return nc


# ------------------------------------------------------------------ runner
def _in_maps(plan):
    st = plan["struct"]
    maps = []
    for c in range(N_CORES):
        pc = plan["per_core"][c]
        m = dict(
            idx1=pc["idx1"], dcol1=pc["dcol1"], drow1=pc["drow1"], ee1=pc["ee1"],
            idx2=pc["idx2"], dcol2=pc["dcol2"], drow2=pc["drow2"], ee2=pc["ee2"],
            embT_src=pc["embT_src"], embT_dst=pc["embT_dst"],
            d2idx=pc["d2idx"], hA=pc["hA"], hB=pc["hB"],
        )
        m.update(plan["shared"])
        maps.append({k: np.ascontiguousarray(v) for k, v in m.items()})
    return maps


def run_device(plan, dbg=False, trace=False):
    from concourse.bass_utils import run_bass_kernel_spmd
    key = (tuple(sorted(plan["struct"].items())), dbg)
    if key not in _NC_CACHE:
        _NC_CACHE[key] = _build_nc(plan["struct"], dbg=dbg)
    nc = _NC_CACHE[key]
    maps = _in_maps(plan)
    br = run_bass_kernel_spmd(nc, maps, list(range(N_CORES)), trace=trace)
    return br


def _assemble(plan, results):
    outs = np.concatenate([results[c]["out_logits"] for c in range(N_CORES)],
                          axis=0)
    outs = outs[:plan["npairs"]]
    bsz = plan["bsz"]
    pos_logit = outs[:bsz].astype(np.float32)
    neg_logits = outs[bsz:].astype(np.float32)
    return pos_logit, neg_logits


def kernel(**inputs):
    plan = _prep(inputs)
    br = run_device(plan)
    return _assemble(plan, br.results)


# convenience for test harnesses
def kernel_traced(**inputs):
    plan = _prep(inputs)
    br = run_device(plan, trace=True)
    return _assemble(plan, br.results), br


# revision 9
# speedup vs baseline: 1.1132x; 1.1132x over previous
"""Simple-HGN (2-layer edge-typed GAT + link-pred head) on 8 trn2 NeuronCores.

Strategy (see spec sharding_hint): destinations are sharded across the 8
cores; edges are partitioned by destination owner so segment softmax /
segment sums are core-local; the per-layer node feature tables
([feat | el] rows) are exchanged with AllGather; small weights replicated.

Device-side structure (identical SPMD program on all cores; all per-core
variation lives in input data):
  - gather table rows: [feat(128) | el(4) | pad] fp32, 192-elem (768B) stride.
  - 4 gather "groups" = row ranges <= 32767 (dma_gather idx is int16),
    issued on 4 SWDGE queues.
  - destination slots: B blocks x 128 slots per core. Per (block, group):
    TBG=2 tiles of <=128 edges, segments never split across tiles.
  - per tile: one-hot [edge,slot] + transposed one-hot via DVE is_equal;
    er-expansion matmul; fused aggregation+denominator matmul with
    rhs [V | a] where V = a * feat(src), a = exp(leaky(el+er+ee)).
  - per block: x = agg * recip(denom + 1e-16) + resid (+relu for layer 1).
  - Head: pair gathers from the AllGather'd final features + tiny matmuls.
"""

import os
import numpy as np

# ---------------------------------------------------------------- constants
N_CORES = 8
HEADS, HID, IN_CH, EDGE_CH = 4, 32, 128, 32
D = HEADS * HID            # 128
TROW = 192                 # table row stride in f32 elems (768B)
G = 4                      # gather groups
TBG = 2                    # tiles per (block, group)
SBB = 4                    # blocks per superblock (gather batch)
BT = TBG * SBB             # tiles per gather batch per group (8 -> 1024 idxs)
NIDX = BT * 128            # idxs per gather
PAD_EE = -500.0            # pad-edge ee -> exp(leaky(PAD_EE)) == 0

_NC_CACHE = {}


# ------------------------------------------------------------ host planning
def _fold_weights(W, al, ar):
    Wf = W.astype(np.float64).reshape(W.shape[0], HEADS, HID)
    Wal = np.einsum("chd,hd->ch", Wf, al.astype(np.float64)).astype(np.float32)
    War = np.einsum("chd,hd->ch", Wf, ar.astype(np.float64)).astype(np.float32)
    return Wal, War


def _ee_table(eemb, We, ae):
    ef = (eemb.astype(np.float64) @ We.astype(np.float64)).reshape(
        eemb.shape[0], HEADS, EDGE_CH)
    return np.einsum("thd,hd->th", ef, ae.astype(np.float64)).astype(np.float32)


def _wrap_idx(flat):
    """[.., n] int -> [.., 128, n//16] int16 (wrap 16, replicate 8x)."""
    n = flat.shape[-1]
    w = flat.reshape(flat.shape[:-1] + (n // 16, 16))
    w = np.swapaxes(w, -1, -2)  # [.., 16, n//16]
    reps = (1,) * (w.ndim - 2) + (8, 1)
    return np.ascontiguousarray(np.tile(w, reps)).astype(np.int16)


def _balanced_split(deg, n_parts):
    """Split positions [0..len(deg)) into n_parts contiguous ranges with
    roughly equal sum(deg). Returns boundary list of len n_parts+1."""
    cum = np.cumsum(deg)
    total = int(cum[-1]) if len(cum) else 0
    bounds = [0]
    for c in range(1, n_parts):
        t = total * c / n_parts
        bounds.append(int(np.searchsorted(cum, t)))
    bounds.append(len(deg))
    # enforce monotone
    for i in range(1, len(bounds)):
        bounds[i] = max(bounds[i], bounds[i - 1])
    return bounds


class _LayerPlan:
    """Per-layer, per-core tiling plan + streams."""

    def __init__(self, n_dst, n_edges, B):
        self.B = B  # blocks (pre-common-pad)
        self.slot_of = np.full(n_dst, -1, np.int64)     # core-local dst -> slot
        self.edge_tile = np.zeros((n_edges, 4), np.int64)  # (b, g, t, fill)


def _assemble_core(dst_pos, e_dst_pos, e_grp, tbg):
    """Greedy block/tile assembly for one core.

    dst_pos: core-local dst indices 0..nd-1 (sorted unique positions).
    e_dst_pos: per edge core-local dst position (edges sorted by (dst, grp)).
    e_grp: per edge group.
    Returns _LayerPlan (B, slot_of, edge_tile).
    """
    nd = len(dst_pos)
    ne = len(e_dst_pos)
    # per (dst, group) edge counts and start offsets (edges sorted by dst,grp)
    cnt = np.zeros((nd, G), np.int64)
    np.add.at(cnt, (e_dst_pos, e_grp), 1)
    starts = np.zeros((nd, G), np.int64)
    flat = cnt.reshape(-1)
    starts.reshape(-1)[1:] = np.cumsum(flat)[:-1]

    plan = _LayerPlan(nd, ne, 0)
    b = 0
    npos = 0
    fill = [0] * G
    cur = [0] * G
    for p in range(nd):
        c = cnt[p]
        ok = npos < 128
        if ok:
            for g in range(G):
                if c[g] == 0:
                    continue
                if fill[g] + c[g] > 128 and cur[g] + 1 >= tbg:
                    ok = False
                    break
        if not ok:
            b += 1
            npos = 0
            fill = [0] * G
            cur = [0] * G
        plan.slot_of[p] = b * 128 + npos
        npos += 1
        for g in range(G):
            if c[g] == 0:
                continue
            if fill[g] + c[g] > 128:
                cur[g] += 1
                fill[g] = 0
            s = starts[p, g]
            plan.edge_tile[s:s + c[g], 0] = b
            plan.edge_tile[s:s + c[g], 1] = g
            plan.edge_tile[s:s + c[g], 2] = cur[g]
            plan.edge_tile[s:s + c[g], 3] = fill[g] + np.arange(c[g])
            fill[g] += c[g]
    plan.B = b + 1
    return plan


def _layer_streams(plan, B_pad, e_srcloc, e_ee, e_dloc):
    """Build idx / dcol / drow / ee stream arrays for one core & layer.

    e_srcloc: group-local gather row per edge (int), e_ee: [ne,4] f32,
    e_dloc: slot - 128*b per edge.
    """
    NSB = B_pad // SBB
    idx = np.zeros((NSB, G, NIDX), np.int64)
    # meta[..., 0:4] = ee, meta[..., 4] = dstlocal (one DMA per superblock)
    meta = np.zeros((NSB, 128, G, BT, 5), np.float32)
    meta[..., 0:4] = PAD_EE
    drow = np.zeros((NSB, G, BT, 128), np.float32)

    b = plan.edge_tile[:, 0]
    g = plan.edge_tile[:, 1]
    t = plan.edge_tile[:, 2]
    k = plan.edge_tile[:, 3]
    sb = b // SBB
    j = (b % SBB) * TBG + t
    i = j * 128 + k
    idx[sb, g, i] = e_srcloc
    meta[sb, k, g, j, 4] = e_dloc
    drow[sb, g, j, k] = e_dloc
    meta[sb, k, g, j, 0:4] = e_ee
    wrapped = _wrap_idx(idx.reshape(NSB, G, NIDX))          # [NSB,G,128,64]
    wrapped = np.ascontiguousarray(wrapped.transpose(0, 2, 1, 3))  # [NSB,128,G,64]
    return wrapped, meta, drow


def _prep(inputs):
    """Full host preprocessing -> plan dict (structure consts + per-core arrays)."""
    ei = np.asarray(inputs["edge_index"])
    src_all = np.asarray(ei[0]).astype(np.int64)
    dst_all = np.asarray(ei[1]).astype(np.int64)
    et_all = np.asarray(inputs["edge_type"]).astype(np.int64)
    n_id = np.asarray(inputs["n_id"]).astype(np.int64)
    neg = int(np.asarray(inputs["neg_sample_num"]))
    id_emb = np.asarray(inputs["id_emb"]).astype(np.float32)
    N = id_emb.shape[0]

    Wal0, War0 = _fold_weights(inputs["W0"], inputs["al0"], inputs["ar0"])
    Wal1, War1 = _fold_weights(inputs["W1"], inputs["al1"], inputs["ar1"])
    ee0 = _ee_table(np.asarray(inputs["eemb0"]), np.asarray(inputs["We0"]),
                    np.asarray(inputs["ae0"]))
    ee1 = _ee_table(np.asarray(inputs["eemb1"]), np.asarray(inputs["We1"]),
                    np.asarray(inputs["ae1"]))

    bsz = len(n_id) // (2 + neg)

    # ---- pruning ----
    T2 = np.unique(n_id)
    m2 = np.isin(dst_all, T2)
    S2 = np.unique(src_all[m2])
    T1 = np.union1d(T2, S2)
    m1 = np.isin(dst_all, T1)
    S1 = np.unique(src_all[m1])

    # ---- L1 structures ----
    # gather table rows = S1 positions, sharded into 8 producer chunks.
    # Each chunk is split in 4 quarters; quarter q of all producers is
    # AllGather'd into its own table tensor = gather group q.
    PROWS = -(-len(S1) // N_CORES)
    PROWS = -(-PROWS // 512) * 512
    PQ1 = PROWS // 4
    GR1 = N_CORES * PQ1  # rows per group table
    assert GR1 <= 32767, GR1

    src1 = src_all[m1]
    dst1 = dst_all[m1]
    et1 = et_all[m1]
    e_spos1 = np.searchsorted(S1, src1)
    RP = -(-len(S1) // N_CORES)  # real rows per producer (last may be short)
    prod = e_spos1 // RP
    loc = e_spos1 - prod * RP
    e_grp1 = loc // PQ1
    e_sloc1 = prod * PQ1 + (loc - e_grp1 * PQ1)

    # dst cores: contiguous, edge-balanced over T1
    t1pos = np.searchsorted(T1, dst1)
    deg1 = np.bincount(t1pos, minlength=len(T1))
    cb1 = _balanced_split(deg1, N_CORES)

    # ---- per-core L1 assembly ----
    order1 = np.lexsort((e_grp1, t1pos))
    t1pos_s, grp1_s, sloc1_s = t1pos[order1], e_grp1[order1], e_sloc1[order1]
    et1_s, src1_s = et1[order1], src1[order1]
    plans1 = []
    for c in range(N_CORES):
        lo, hi = cb1[c], cb1[c + 1]
        emask = (t1pos_s >= lo) & (t1pos_s < hi)
        plans1.append((np.nonzero(emask)[0], lo, hi))
    cores1 = []
    B1 = 1
    for c in range(N_CORES):
        eidx, lo, hi = plans1[c]
        pl = _assemble_core(np.arange(hi - lo), t1pos_s[eidx] - lo,
                            grp1_s[eidx], TBG)
        cores1.append((pl, eidx, lo, hi))
        B1 = max(B1, pl.B)
    B1 = -(-B1 // SBB) * SBB
    NSB1 = B1 // SBB

    # global L1 slot of every T1 node (for L2 src mapping & residual/er)
    slot1_of_t1 = np.full(len(T1), -1, np.int64)
    for c in range(N_CORES):
        pl, eidx, lo, hi = cores1[c]
        slot1_of_t1[lo:hi] = c * B1 * 128 + pl.slot_of

    # ---- L2 structures ----
    src2 = src_all[m2]
    dst2 = dst_all[m2]
    et2 = et_all[m2]
    # gather position in table2 = global L1 slot of src (src in T1 since T1>=S2)
    gpos2 = slot1_of_t1[np.searchsorted(T1, src2)]
    PB1 = B1 * 128 // 4
    GR2 = N_CORES * PB1
    assert GR2 <= 32767, GR2
    c2 = gpos2 // (B1 * 128)
    sl = gpos2 - c2 * (B1 * 128)
    e_grp2 = sl // PB1
    e_sloc2 = c2 * PB1 + (sl - e_grp2 * PB1)

    # L2 dst cores: T2 split at the same T1 ownership boundaries
    t2_in_t1 = np.searchsorted(T1, T2)
    t2_core = np.searchsorted(np.array(cb1[1:]), t2_in_t1, side="right")
    t2pos = np.searchsorted(T2, dst2)
    cores2 = []
    B2 = 1
    order2 = np.lexsort((e_grp2, t2pos))
    t2pos_s, grp2_s, sloc2_s = t2pos[order2], e_grp2[order2], e_sloc2[order2]
    et2_s = et2[order2]
    cb2 = [int(np.searchsorted(t2_core, c)) for c in range(N_CORES)] + [len(T2)]
    for c in range(N_CORES):
        lo, hi = cb2[c], cb2[c + 1]
        emask = (t2pos_s >= lo) & (t2pos_s < hi)
        eidx = np.nonzero(emask)[0]
        pl = _assemble_core(np.arange(hi - lo), t2pos_s[eidx] - lo,
                            grp2_s[eidx], TBG)
        cores2.append((pl, eidx, lo, hi))
        B2 = max(B2, pl.B)
    B2 = -(-B2 // SBB) * SBB
    NSB2 = B2 // SBB

    slot2_of_t2 = np.full(len(T2), -1, np.int64)
    for c in range(N_CORES):
        pl, eidx, lo, hi = cores2[c]
        slot2_of_t2[lo:hi] = c * B2 * 128 + pl.slot_of

    # ---- per-core input arrays ----
    per_core = []
    for c in range(N_CORES):
        pl1, eidx1, lo1, hi1 = cores1[c]
        pl2, eidx2, lo2, hi2 = cores2[c]

        idx1, meta1, drow1 = _layer_streams(
            pl1, B1, sloc1_s[eidx1], ee0[et1_s[eidx1]],
            (pl1.slot_of[t1pos_s[eidx1] - lo1] % 128).astype(np.float32))
        idx2, meta2, drow2 = _layer_streams(
            pl2, B2, sloc2_s[eidx2], ee1[et2_s[eidx2]],
            (pl2.slot_of[t2pos_s[eidx2] - lo2] % 128).astype(np.float32))

        # producer chunk sources (S1 rows of this producer)
        s1_lo, s1_hi = c * RP, min((c + 1) * RP, len(S1))
        embT_src = np.zeros((IN_CH, PROWS), np.float32)
        embT_src[:, : s1_hi - s1_lo] = id_emb[S1[s1_lo:s1_hi]].T

        # dst-side id_emb expanded to slot space
        embT_dst = np.zeros((IN_CH, B1 * 128), np.float32)
        sl = pl1.slot_of
        embT_dst[:, sl] = id_emb[T1[lo1:hi1]].T

        # L2 dst slabs: L1 slot (core-local) of each L2 dst slot
        d2idx = np.zeros((B2, 128), np.int64)
        g1 = slot1_of_t1[t2_in_t1[lo2:hi2]] - c * B1 * 128
        assert (g1 >= 0).all() and (g1 < B1 * 128).all()
        sl2 = pl2.slot_of
        d2idx[sl2 // 128, sl2 % 128] = g1
        d2w = _wrap_idx(d2idx.reshape(B2, 128))

        per_core.append(dict(
            idx1=idx1, meta1=meta1, drow1=drow1,
            idx2=idx2, meta2=meta2, drow2=drow2,
            embT_src=embT_src, embT_dst=embT_dst, d2idx=d2w,
        ))

    # ---- head pairs ----
    out_n = n_id[:bsz]
    pos_n = n_id[bsz:2 * bsz]
    neg_n = n_id[2 * bsz:]
    a_nodes = np.concatenate([out_n, np.repeat(out_n, neg)])
    b_nodes = np.concatenate([pos_n, neg_n])
    a_slot = slot2_of_t2[np.searchsorted(T2, a_nodes)]
    b_slot = slot2_of_t2[np.searchsorted(T2, b_nodes)]
    npairs = len(a_nodes)
    HB_total = -(-npairs // (128 * N_CORES)) * N_CORES
    HB = HB_total // N_CORES
    pad_n = HB_total * 128 - npairs
    a_slot = np.concatenate([a_slot, np.zeros(pad_n, np.int64)])
    b_slot = np.concatenate([b_slot, np.zeros(pad_n, np.int64)])
    a_slot = a_slot.reshape(N_CORES, HB * 128)
    b_slot = b_slot.reshape(N_CORES, HB * 128)
    for c in range(N_CORES):
        per_core[c]["hA"] = _wrap_idx(a_slot[c].reshape(HB, 128))
        per_core[c]["hB"] = _wrap_idx(b_slot[c].reshape(HB, 128))

    # ---- shared weight inputs ----
    f32 = np.float32
    shared = dict(
        Wsrc1=np.concatenate([np.asarray(inputs["W0"], f32), Wal0], axis=1),
        Wdst1=np.concatenate([np.asarray(inputs["Wres0"], f32), War0], axis=1),
        Wsrc2=np.concatenate([np.asarray(inputs["W1"], f32), Wal1,
                              np.asarray(inputs["Wres1"], f32), War1], axis=1),
        pW1a=np.asarray(inputs["pW1"], f32)[:D],
        pW1b=np.asarray(inputs["pW1"], f32)[D:],
        pW2=np.asarray(inputs["pW2"], f32),
        pb1=np.asarray(inputs["pb1"], f32).reshape(HID, 1),
        pb2=np.full((128, 1), float(np.asarray(inputs["pb2"])[0]), f32),
        iota_row=np.tile(np.arange(128, dtype=f32)[None, :], (128, 1)),
        iota_col=np.arange(128, dtype=f32).reshape(128, 1),
        ident=np.eye(128, dtype=f32),
    )

    struct = dict(PROWS=PROWS, GR1=GR1, B1=B1, NSB1=NSB1, B2=B2, NSB2=NSB2,
                  GR2=GR2, HB=HB)
    return dict(struct=struct, per_core=per_core, shared=shared,
                bsz=bsz, neg=neg, npairs=npairs)


# ------------------------------------------------------------- bass builder
def _build_nc(st, dbg=False):
    import concourse.bass as bass
    import concourse.bacc as bacc
    import concourse.mybir as mybir
    import concourse.tile as tile
    from concourse import library_config

    f32 = mybir.dt.float32
    i16 = mybir.dt.int16
    AF = mybir.ActivationFunctionType
    ALU = mybir.AluOpType

    PROWS, GR1, B1, NSB1 = st["PROWS"], st["GR1"], st["B1"], st["NSB1"]
    B2, NSB2, GR2, HB = st["B2"], st["NSB2"], st["GR2"], st["HB"]

    nc = bacc.Bacc("TRN2", num_devices=N_CORES, num_swdge_queues=G)

    # -------- dram tensors
    def din(name, shape, dt=f32):
        return nc.dram_tensor(name, shape, dt, kind="ExternalInput")

    idx1 = din("idx1", [NSB1, 128, G, NIDX // 16], i16)
    meta1 = din("meta1", [NSB1, 128, G, BT, 5])
    drow1 = din("drow1", [NSB1, G, BT, 128])
    idx2 = din("idx2", [NSB2, 128, G, NIDX // 16], i16)
    meta2 = din("meta2", [NSB2, 128, G, BT, 5])
    drow2 = din("drow2", [NSB2, G, BT, 128])
    embT_src = din("embT_src", [IN_CH, PROWS])
    embT_dst = din("embT_dst", [IN_CH, B1 * 128])
    d2idx = din("d2idx", [B2, 128, 8], i16)
    hA = din("hA", [HB, 128, 8], i16)
    hB = din("hB", [HB, 128, 8], i16)
    Wsrc1 = din("Wsrc1", [IN_CH, 132])
    Wdst1 = din("Wdst1", [IN_CH, 132])
    Wsrc2 = din("Wsrc2", [D, 264])
    pW1a = din("pW1a", [D, HID])
    pW1b = din("pW1b", [D, HID])
    pW2 = din("pW2", [HID, 1])
    pb1 = din("pb1", [HID, 1])
    pb2 = din("pb2", [128, 1])
    iota_row = din("iota_row", [128, 128])
    iota_col = din("iota_col", [128, 1])
    ident = din("ident", [128, 128])

    out_logits = nc.dram_tensor("out_logits", [HB * 128, 1], f32,
                                kind="ExternalOutput")
    if dbg:
        dbg_x1 = nc.dram_tensor("dbg_x1", [B1 * 128, D], f32,
                                kind="ExternalOutput")
        dbg_x2 = nc.dram_tensor("dbg_x2", [B2 * 128, D], f32,
                                kind="ExternalOutput")

    PQ1 = PROWS // 4
    PB1 = B1 * 128 // 4
    chunk1 = nc.dram_tensor("chunk1", [PROWS, TROW], f32)
    table1q = [nc.dram_tensor(f"table1q{q}", [N_CORES * PQ1, TROW], f32,
                              addr_space="Shared") for q in range(G)]
    chunk2 = nc.dram_tensor("chunk2", [B1 * 128, TROW], f32)
    table2q = [nc.dram_tensor(f"table2q{q}", [N_CORES * PB1, TROW], f32,
                              addr_space="Shared") for q in range(G)]
    dst2t = nc.dram_tensor("dst2t", [B1 * 128, TROW], f32)
    x2chunk = nc.dram_tensor("x2chunk", [B2 * 128, D], f32)
    x2ag = nc.dram_tensor("x2ag", [N_CORES * B2 * 128, D], f32,
                          addr_space="Shared")

    rg = [list(range(N_CORES))]

    with tile.TileContext(nc) as tc:
        nc.gpsimd.load_library(library_config.mlp)
        import contextlib
        ctx = contextlib.ExitStack()
        cpool = ctx.enter_context(tc.tile_pool(name="const", bufs=1))
        slab = ctx.enter_context(tc.tile_pool(name="slab", bufs=1))
        sp = ctx.enter_context(tc.tile_pool(name="stream", bufs=4))
        gp = ctx.enter_context(tc.tile_pool(name="gbuf", bufs=5))
        ohp = ctx.enter_context(tc.tile_pool(name="oh", bufs=6))
        ohtp = ctx.enter_context(tc.tile_pool(name="oht", bufs=4))
        vp = ctx.enter_context(tc.tile_pool(name="vsl", bufs=4))
        smp = ctx.enter_context(tc.tile_pool(name="small", bufs=10))
        pp_agg = ctx.enter_context(tc.tile_pool(name="ps_agg", bufs=3, space="PSUM"))
        pp_er = ctx.enter_context(tc.tile_pool(name="ps_er", bufs=2, space="PSUM"))
        pp_prod = ctx.enter_context(tc.tile_pool(name="ps_prod", bufs=2, space="PSUM"))
        pp_tp = ctx.enter_context(tc.tile_pool(name="ps_tp", bufs=1, space="PSUM"))

        # -------- consts to SBUF
        def const_tile(src_ap, shape, tag, dt=f32):
            t = cpool.tile(shape, dt, tag=tag)
            nc.sync.dma_start(out=t[:], in_=src_ap)
            return t

        iota_row_t = const_tile(iota_row[:], [128, 128], "c_ir")
        iota_col_t = const_tile(iota_col[:], [128, 1], "c_ic")
        ident_t = const_tile(ident[:], [128, 128], "c_id")
        Wsrc1_t = const_tile(Wsrc1[:], [IN_CH, 132], "c_w1")
        Wdst1_t = const_tile(Wdst1[:], [IN_CH, 132], "c_wd")
        Wsrc2_t = const_tile(Wsrc2[:], [D, 264], "c_w2")
        pW1a_t = const_tile(pW1a[:], [D, HID], "c_pa")
        pW1b_t = const_tile(pW1b[:], [D, HID], "c_pb")
        pW2_t = const_tile(pW2[:], [HID, 1], "c_p2")
        pb1_t = const_tile(pb1[:], [HID, 1], "c_b1")
        pb2_t = const_tile(pb2[:], [128, 1], "c_b2")

        # -------- persistent slabs
        er1_s = slab.tile([128, B1, 4], f32, tag="er1")
        rs1_s = slab.tile([128, B1, D], f32, tag="rs1")
        x1_s = slab.tile([128, B1, D], f32, tag="x1")
        er2_s = slab.tile([128, B2, 4], f32, tag="er2")
        rs2_s = slab.tile([128, B2, D], f32, tag="rs2")
        x2_s = slab.tile([128, B2, D], f32, tag="x2")

        # ---------------- produce layer 1 table (quarter-pipelined) ----------
        def produce_rows(n_tiles, src_of, rhs_t, sink, base):
            i = 0
            while i < n_tiles:
                nb = min(4, n_tiles - i)
                stg = sp.tile([128, 4, 132], f32, tag="pstg")
                for u in range(i, i + nb):
                    emb_t = sp.tile([128, 128], f32, tag="emb")
                    nc.sync.dma_start(out=emb_t[:], in_=src_of(base + u))
                    ps = pp_prod.tile([128, 264], f32, tag="prod")
                    nc.tensor.matmul(out=ps[:, :132], lhsT=emb_t[:], rhs=rhs_t[:],
                                     start=True, stop=True)
                    nc.vector.tensor_copy(out=stg[:, u - i, :], in_=ps[:, :132])
                r0 = (base + i) * 128
                dst = sink[r0:r0 + nb * 128, 0:132].rearrange(
                    "(a p) c -> p a c", p=128)
                nc.scalar.dma_start(out=dst, in_=stg[:, :nb, :])
                i += nb

        for q in range(G):
            produce_rows(PQ1 // 128,
                         lambda u: embT_src[:, u * 128:(u + 1) * 128],
                         Wsrc1_t, chunk1, q * (PQ1 // 128))
            nc.gpsimd.collective_compute(
                "AllGather", ALU.bypass, replica_groups=rg,
                ins=[chunk1[q * PQ1:(q + 1) * PQ1, :]], outs=[table1q[q][:]])
        # dst-side: resid + er slabs
        for b in range(B1):
            emb_t = sp.tile([128, 128], f32, tag="emb")
            nc.sync.dma_start(out=emb_t[:], in_=embT_dst[:, b * 128:(b + 1) * 128])
            ps = pp_prod.tile([128, 264], f32, tag="prod")
            nc.tensor.matmul(out=ps[:, :132], lhsT=emb_t[:], rhs=Wdst1_t[:],
                             start=True, stop=True)
            nc.vector.tensor_copy(out=rs1_s[:, b, :], in_=ps[:, 0:D])
            nc.vector.tensor_copy(out=er1_s[:, b, :], in_=ps[:, D:132])

        # ---------------- generic aggregation layer ----------------
        def agg_layer(NSB, idx_d, meta_d, drow_d, tables, er_slab, rs_slab,
                      x_slab, relu):
            for sb in range(NSB):
                gbufs, vs = [], []
                ohsl_g, mt_g = [], None
                # merged streams: one idx DMA + one meta DMA per superblock
                it = sp.tile([128, G, NIDX // 16], i16, tag="idx")
                nc.sync.dma_start(out=it[:], in_=idx_d[sb])
                mt = sp.tile([128, G, BT, 5], f32, tag="meta")
                nc.sync.dma_start(out=mt[:], in_=meta_d[sb])
                for g in range(G):
                    gb = gp.tile([128, BT, TROW], f32, tag="gb")
                    nc.gpsimd.dma_gather(
                        gb[:], tables[g][:], it[:, g, :],
                        NIDX, NIDX, TROW, single_packet=False, queue_num=g)
                    gbufs.append(gb)
                    dr = sp.tile([128, BT, 128], f32, tag="drow")
                    drsrc = drow_d[sb, g]  # [BT, 128]
                    bcast = bass.AP(drsrc.tensor, drsrc.offset,
                                    [[0, 128]] + list(drsrc.ap))
                    nc.sync.dma_start(out=dr[:], in_=bcast)

                    # slab one-hot builds (single TT each)
                    ohsl = ohp.tile([128, BT, 128], f32, tag="oh")
                    dc_ap = mt[:, g, :, 4:5]  # [128, BT, 1]
                    dc_b = bass.AP(dc_ap.tensor, dc_ap.offset,
                                   list(dc_ap.ap)[:-1] + [[0, 128]])
                    ir_ap = iota_row_t[:]
                    ir_b = bass.AP(ir_ap.tensor, ir_ap.offset,
                                   [list(ir_ap.ap)[0], [0, BT],
                                    list(ir_ap.ap)[1]])
                    nc.vector.tensor_tensor(out=ohsl[:], in0=ir_b, in1=dc_b,
                                            op=ALU.is_equal)
                    ohsl_g.append(ohsl)
                    ohtsl = ohtp.tile([128, BT, 128], f32, tag="oht")
                    ic_ap = iota_col_t[:]
                    ic_b = bass.AP(ic_ap.tensor, ic_ap.offset,
                                   [list(ic_ap.ap)[0], [0, BT], [0, 128]])
                    nc.vector.tensor_tensor(out=ohtsl[:], in0=dr[:], in1=ic_b,
                                            op=ALU.is_equal)

                    er_ps = pp_er.tile([128, BT * 4], f32, tag="er")
                    for j in range(BT):
                        b = sb * SBB + j // TBG
                        nc.tensor.matmul(out=er_ps[:, 4 * j:4 * j + 4],
                                         lhsT=ohtsl[:, j, :],
                                         rhs=er_slab[:, b, :],
                                         start=True, stop=True)
                    # batched logit pipeline
                    v = vp.tile([128, BT, 132], f32, tag="v")
                    vs.append(v)
                    eet = sp.tile([128, BT, 4], f32, tag="ee")
                    nc.vector.tensor_tensor(
                        out=eet[:], in0=mt[:, g, :, 0:4],
                        in1=gb[:, :, 128:132], op=ALU.add)
                    nc.vector.tensor_tensor(
                        out=v[:, :, 128:132], in0=eet[:],
                        in1=er_ps[:].rearrange("p (j h) -> p j h", h=4),
                        op=ALU.add)
                    nc.vector.tensor_scalar(
                        out=eet[:], in0=v[:, :, 128:132], scalar1=0.2,
                        scalar2=None, op0=ALU.mult)
                    nc.vector.tensor_tensor(
                        out=v[:, :, 128:132], in0=v[:, :, 128:132],
                        in1=eet[:], op=ALU.max)
                    nc.scalar.activation(out=v[:, :, 128:132],
                                         in_=v[:, :, 128:132], func=AF.Exp)
                    nc.vector.tensor_tensor(
                        out=v[:, :, 0:D].rearrange("p j (h d) -> p j h d", d=HID),
                        in0=gb[:, :, 0:D].rearrange("p j (h d) -> p j h d", d=HID),
                        in1=v[:, :, 128:132].to_broadcast([128, BT, 4, HID]),
                        op=ALU.mult)

                for bl in range(SBB):
                    b = sb * SBB + bl
                    agg = pp_agg.tile([128, 132], f32, tag="agg")
                    n_mm = G * TBG
                    m = 0
                    for g in range(G):
                        for t in range(TBG):
                            j = bl * TBG + t
                            nc.tensor.matmul(
                                out=agg[:], lhsT=ohsl_g[g][:, j, :],
                                rhs=vs[g][:, j, :],
                                start=(m == 0), stop=(m == n_mm - 1))
                            m += 1
                    den = smp.tile([128, 4], f32, tag="den")
                    nc.vector.tensor_scalar(out=den[:], in0=agg[:, 128:132],
                                            scalar1=1e-16, scalar2=None,
                                            op0=ALU.add)
                    rec = smp.tile([128, 4], f32, tag="rec")
                    nc.vector.reciprocal(out=rec[:], in_=den[:])
                    xs = smp.tile([128, D], f32, tag="xstg")
                    nc.vector.tensor_tensor(
                        out=xs[:].rearrange("p (h d) -> p h d", d=HID),
                        in0=agg[:, 0:D].rearrange("p (h d) -> p h d", d=HID),
                        in1=rec[:].to_broadcast([128, 4, HID]), op=ALU.mult)
                    if relu:
                        nc.vector.tensor_tensor(out=xs[:], in0=xs[:],
                                                in1=rs_slab[:, b, :], op=ALU.add)
                        nc.scalar.activation(out=x_slab[:, b, :], in_=xs[:],
                                             func=AF.Relu)
                    else:
                        nc.vector.tensor_tensor(out=x_slab[:, b, :], in0=xs[:],
                                                in1=rs_slab[:, b, :], op=ALU.add)

        agg_layer(NSB1, idx1, meta1, drow1, table1q,
                  er1_s, rs1_s, x1_s, relu=True)
        if dbg:
            for b in range(B1):
                nc.scalar.dma_start(out=dbg_x1[b * 128:(b + 1) * 128, :],
                                    in_=x1_s[:, b, :])

        # ---------------- produce layer 2 table + dst slabs ----------------
        BQ = B1 // G  # blocks per quarter (B1 multiple of 4: PB1 = BQ*128)
        assert BQ * G == B1
        for q in range(G):
            b0 = q * BQ
            b = b0
            while b < b0 + BQ:
                nb = min(2, b0 + BQ - b)
                stg = sp.tile([128, 2, 132], f32, tag="pstg")
                stg2 = sp.tile([128, 2, 132], f32, tag="pstg")
                for u in range(b, b + nb):
                    tp = pp_tp.tile([128, 128], f32, tag="tp")
                    nc.tensor.transpose(out=tp[:], in_=x1_s[:, u, :],
                                        identity=ident_t[:])
                    xt = sp.tile([128, 128], f32, tag="xt")
                    nc.vector.tensor_copy(out=xt[:], in_=tp[:])
                    ps = pp_prod.tile([128, 264], f32, tag="prod")
                    nc.tensor.matmul(out=ps[:], lhsT=xt[:], rhs=Wsrc2_t[:],
                                     start=True, stop=True)
                    nc.vector.tensor_copy(out=stg[:, u - b, :], in_=ps[:, 0:132])
                    nc.vector.tensor_copy(out=stg2[:, u - b, :],
                                          in_=ps[:, 132:264])
                dst = chunk2[b * 128:(b + nb) * 128, 0:132].rearrange(
                    "(a p) c -> p a c", p=128)
                nc.scalar.dma_start(out=dst, in_=stg[:, :nb, :])
                dst2 = dst2t[b * 128:(b + nb) * 128, 0:132].rearrange(
                    "(a p) c -> p a c", p=128)
                nc.scalar.dma_start(out=dst2, in_=stg2[:, :nb, :])
                b += nb
            nc.gpsimd.collective_compute(
                "AllGather", ALU.bypass, replica_groups=rg,
                ins=[chunk2[q * PB1:(q + 1) * PB1, :]], outs=[table2q[q][:]])

        # L2 dst slabs via gather from dst2t
        for b2 in range(B2):
            it = sp.tile([128, 8], i16, tag="idx")
            nc.sync.dma_start(out=it[:], in_=d2idx[b2])
            db = gp.tile([128, 1, TROW], f32, tag="d2b")
            nc.gpsimd.dma_gather(db[:], dst2t[:], it[:], 128, 128, TROW,
                                 single_packet=False, queue_num=b2 % G)
            nc.vector.tensor_copy(out=rs2_s[:, b2, :], in_=db[:, 0, 0:D])
            nc.vector.tensor_copy(out=er2_s[:, b2, :], in_=db[:, 0, D:132])

        agg_layer(NSB2, idx2, meta2, drow2, table2q,
                  er2_s, rs2_s, x2_s, relu=False)
        if dbg:
            for b in range(B2):
                nc.scalar.dma_start(out=dbg_x2[b * 128:(b + 1) * 128, :],
                                    in_=x2_s[:, b, :])

        # ---------------- x2 AllGather + head ----------------
        for b2 in range(B2):
            nc.scalar.dma_start(out=x2chunk[b2 * 128:(b2 + 1) * 128, :],
                                in_=x2_s[:, b2, :])
        nc.gpsimd.collective_compute(
            "AllGather", ALU.bypass, replica_groups=rg,
            ins=[x2chunk[:]], outs=[x2ag[:]])

        for hb in range(HB):
            gtiles = []
            for nm, idxd in (("A", hA), ("B", hB)):
                it = sp.tile([128, 8], i16, tag="idx")
                nc.sync.dma_start(out=it[:], in_=idxd[hb])
                gt = gp.tile([128, 1, D], f32, tag="hg")
                nc.gpsimd.dma_gather(gt[:], x2ag[:], it[:], 128, 128, D,
                                     single_packet=False,
                                     queue_num=(2 * hb + (nm == "B")) % G)
                tp = pp_tp.tile([128, 128], f32, tag="tp")
                nc.tensor.transpose(out=tp[:], in_=gt[:, 0, :],
                                    identity=ident_t[:])
                ts = sp.tile([128, 128], f32, tag="xt")
                nc.vector.tensor_copy(out=ts[:], in_=tp[:])
                gtiles.append(ts)
            zp = pp_prod.tile([HID, 128], f32, tag="prod")
            nc.tensor.matmul(out=zp[:], lhsT=pW1a_t[:], rhs=gtiles[0][:],
                             start=True, stop=False)
            nc.tensor.matmul(out=zp[:], lhsT=pW1b_t[:], rhs=gtiles[1][:],
                             start=False, stop=True)
            zb = sp.tile([HID, 128], f32, tag="zb")
            nc.vector.tensor_scalar(out=zb[:], in0=zp[:], scalar1=pb1_t[:],
                                    scalar2=None, op0=ALU.add)
            zt = sp.tile([HID, 128], f32, tag="zt")
            nc.vector.tensor_scalar(out=zt[:], in0=zb[:], scalar1=0.2,
                                    scalar2=None, op0=ALU.mult)
            zs = sp.tile([HID, 128], f32, tag="zs")
            nc.vector.tensor_tensor(out=zs[:], in0=zb[:], in1=zt[:],
                                    op=ALU.max)
            op = pp_er.tile([128, 1], f32, tag="er")
            nc.tensor.matmul(out=op[:], lhsT=zs[:], rhs=pW2_t[:],
                             start=True, stop=True)
            ot = smp.tile([128, 1], f32, tag="ot")
            nc.vector.tensor_tensor(out=ot[:], in0=op[:], in1=pb2_t[:],
                                    op=ALU.add)
            nc.sync.dma_start(out=out_logits[hb * 128:(hb + 1) * 128, :],
                              in_=ot[:])
        ctx.close()
    nc.compile()
    return nc


# ------------------------------------------------------------------ runner
def _in_maps(plan):
    st = plan["struct"]
    maps = []
    for c in range(N_CORES):
        pc = plan["per_core"][c]
        m = dict(
            idx1=pc["idx1"], meta1=pc["meta1"], drow1=pc["drow1"],
            idx2=pc["idx2"], meta2=pc["meta2"], drow2=pc["drow2"],
            embT_src=pc["embT_src"], embT_dst=pc["embT_dst"],
            d2idx=pc["d2idx"], hA=pc["hA"], hB=pc["hB"],
        )
        m.update(plan["shared"])
        maps.append({k: np.ascontiguousarray(v) for k, v in m.items()})
    return maps


def run_device(plan, dbg=False, trace=False):
    from concourse.bass_utils import run_bass_kernel_spmd
    key = (tuple(sorted(plan["struct"].items())), dbg)
    if key not in _NC_CACHE:
        _NC_CACHE[key] = _build_nc(plan["struct"], dbg=dbg)
    nc = _NC_CACHE[key]
    maps = _in_maps(plan)
    br = run_bass_kernel_spmd(nc, maps, list(range(N_CORES)), trace=trace)
    return br


def _assemble(plan, results):
    outs = np.concatenate([results[c]["out_logits"] for c in range(N_CORES)],
                          axis=0)
    outs = outs[:plan["npairs"]]
    bsz = plan["bsz"]
    pos_logit = outs[:bsz].astype(np.float32)
    neg_logits = outs[bsz:].astype(np.float32)
    return pos_logit, neg_logits


def kernel(**inputs):
    plan = _prep(inputs)
    br = run_device(plan)
    return _assemble(plan, br.results)


# convenience for test harnesses
def kernel_traced(**inputs):
    plan = _prep(inputs)
    br = run_device(plan, trace=True)
    return _assemble(plan, br.results), br
